# revision 6
# baseline (speedup 1.0000x reference)
"""LGCN (K-hop symmetric-normalized graph propagation) on 8 Trainium2 cores.

Algorithm: Z = concat([X, A_hat X, ..., A_hat^K X]) with
A_hat = D^-1/2 (A + I) D^-1/2 (existing self-edges dropped, loops added).

Folding: with dis = deg^-1/2, x'_k = dis * y_k obeys
    x'_{k+1} = dinv * segsum_dst(x'_k[src]),   y_k = x'_k / dis
over the unweighted self-loop-augmented edge list. So each hop is a pure
gather + segment-sum + row-scale: no per-edge weights on device.

Device mapping (SPMD, 8 cores, dst-sharded):
  - x' table [50176, 64] f32 lives in DRAM, rebuilt per hop by AllGather.
    Hop 0's table is also built on device: each core uploads only its own
    6272 rows (f16), converts to f32, and AllGathers - so the host never
    ships the full table 8x.
  - dma_gather (SWDGE) pulls per-edge source rows into SBUF, 128 edges per
    "chunk" (one free column).
  - one-hot S matrices (S^T[e, d] = edge e targets tile-row d) are built on
    DVE with a single is_equal over a broadcast iota row; segment-sum is
    S^T.T @ G on the PE accumulating into PSUM per 128-dst tile.
  - PSUM is scaled by dinv (next-hop x', f32) and by dis*s_k into fp8-e4m3
    y output per tile (per-hop power-of-2 scale s_k keeps values in fp8
    range; host divides back). Hop blocks carry only ~6% of ||Z||^2 (the
    raw-feature block dominates), so fp8 rounding lands at ~7e-3 rel err
    against the 2e-2 gate.
Edges are padded per (tile, src-half) to uniform chunk counts so the
program is identical on all cores (int16 gather indices need a lo/hi
table split at row 25088 / 17408 base).

Host<->device traffic per call (the wall-clock bottleneck over the axon
tunnel): one packed 1.4MB/core input blob + 3.2MB/core donated output
zeros up, 3.2MB/core fp8 y down (~63MB total vs ~330MB for the all-f32
variant). A persistent JAX compilation cache removes the ~1s per-call
XLA rebuild. Device exec is ~6ms for all 8 hops (measured K=1 vs K=8);
the remaining wall is tunnel transfer plus fixed RPC/trace overhead.
"""
import sys
sys.path.insert(0, "/opt/trn_rl_repo")
import math
import os
import numpy as np

os.environ.setdefault("JAX_COMPILATION_CACHE_DIR", "/tmp/jax_comp_cache")
import jax
try:
    jax.config.update("jax_compilation_cache_dir",
                      os.environ["JAX_COMPILATION_CACHE_DIR"])
    jax.config.update("jax_persistent_cache_min_entry_size_bytes", 0)
    jax.config.update("jax_persistent_cache_min_compile_time_secs", 0.0)
except Exception:
    pass

N = 50000
D = 64
K = int(os.environ.get("LGCN_K", "8"))
NC = 8
NSH = N // NC            # 6250 nodes per core
TILES = 49               # 128-dst tiles per core
ROWS = TILES * 128       # 6272 padded rows per core
TAB = NC * ROWS          # 50176 table rows
THRESH = 25088           # src rows below -> lo gather
HI_BASE = 17408          # hi gather table base (TAB - HI_BASE = 32768 rows)
LO_ROWS = 32768
BT = 7                   # tiles per gather batch
NB = TILES // BT         # 7 batches
# per-hop power-of-2 fp8 output scales (y_k*s_k ~ N(0, 4..10), absmax <~110
# vs e4m3 max 240). Hop stds decay 0.25 -> 0.005 then plateau.
S8 = [32.0, 64.0, 256.0, 512.0, 1024.0, 2048.0, 2048.0, 2048.0][:K] + \
     [2048.0] * max(0, K - 8)

_cache = {}
LAST_RUN_S = None


def _preprocess(feature, edge_index):
    f32 = np.float32
    src = edge_index[0].astype(np.int64)
    dst = edge_index[1].astype(np.int64)
    keep = src != dst
    ks, kd = src[keep], dst[keep]
    deg = (np.bincount(ks, minlength=N) + 1).astype(f32)
    dis = (1.0 / np.sqrt(deg)).astype(f32)
    dinv = (dis * dis).astype(f32)

    # balanced node -> (tile, row) assignment per core, by in-degree
    indeg = np.bincount(kd, minlength=N) + 1
    tile_of = np.empty(N, np.int32)
    row_of = np.empty(N, np.int32)
    for c in range(NC):
        nodes = np.arange(c * NSH, (c + 1) * NSH)
        order = nodes[np.argsort(-indeg[nodes], kind="stable")]
        loads = np.zeros(TILES, np.int64)
        counts = np.zeros(TILES, np.int64)
        for n in order:
            cand = np.where(counts < 128, loads, 1 << 60)
            t = int(np.argmin(cand))
            tile_of[n] = t
            row_of[n] = counts[t]
            counts[t] += 1
            loads[t] += indeg[n]
    core_of = (np.arange(N) // NSH).astype(np.int32)
    tpos = core_of * ROWS + tile_of * 128 + row_of      # table position per node

    # augmented edge list (kept edges + self loops), dst-sharded
    es = np.concatenate([ks, np.arange(N)])
    ed = np.concatenate([kd, np.arange(N)])
    srcr = tpos[es].astype(np.int64)
    ecore = core_of[ed]
    etile = tile_of[ed].astype(np.int64)
    erow = row_of[ed].astype(np.int64)
    lo = srcr < THRESH

    # group edges by (core, tile, half); rank within group
    key = (ecore * TILES + etile) * 2 + (~lo)
    order = np.argsort(key, kind="stable")
    skey = key[order]
    counts = np.bincount(skey, minlength=NC * TILES * 2)
    starts = np.concatenate([[0], np.cumsum(counts)[:-1]])
    rank = np.arange(len(order)) - starts[skey]

    L_C = int(math.ceil(counts[0::2].max() / 128))
    H_C = int(math.ceil(counts[1::2].max() / 128))
    T = L_C + H_C
    BC = BT * T                                         # G cols per batch
    TOTC = TILES * T
    TOT = TOTC * 128

    # slot number for each edge (per core)
    sk = skey
    score = sk // (TILES * 2)
    st = (sk // 2) % TILES
    shalf = sk % 2
    b = st // BT
    ti = st % BT
    chunk = rank // 128
    pos = rank % 128
    col_in_batch = np.where(shalf == 0, ti * L_C + chunk,
                            BT * L_C + ti * H_C + chunk)
    col = b * BC + col_in_batch
    slot = col * 128 + pos

    sidx = np.where(shalf == 0, srcr[order], srcr[order] - HI_BASE).astype(np.int16)
    sdoff = erow[order].astype(np.uint8)

    idx_all = np.zeros((NC, TOT), np.int16)
    doff_all = np.full((NC, TOTC, 128), 255, np.uint8)
    idx_all[score, slot] = sidx
    doff_all[score, col, pos] = sdoff

    # wrap idx per gather block (block = batch x half, contiguous slots);
    # single 16-row copy - the kernel replicates it to 128 partitions.
    lo_n = BT * L_C * 128
    hi_n = BT * H_C * 128
    idxw = np.empty((NC, 16, TOT // 16), np.int16)
    blk_cols = []
    off = 0
    for bb in range(NB):
        for half, nn in ((0, lo_n), (1, hi_n)):
            blk = idx_all[:, off:off + nn]              # [NC, nn]
            w = blk.reshape(NC, nn // 16, 16).transpose(0, 2, 1)  # [NC,16,nn/16]
            c0 = off // 16
            idxw[:, :, c0:c0 + nn // 16] = w
            blk_cols.append((c0, nn))
            off += nn

    # per-tile scale columns: dinv [128, TILES]; dis scaled per hop
    # [128, K*TILES] (hop k uses cols [(k-1)*TILES, k*TILES))
    dinv_cols = np.zeros((NC, 128, TILES), f32)
    dis_cols = np.zeros((NC, 128, TILES), f32)
    dinv_cols[core_of, row_of, tile_of] = dinv
    dis_cols[core_of, row_of, tile_of] = dis
    disk_cols = np.concatenate([dis_cols * s for s in S8], axis=2)

    # per-core x'_0 rows in SBUF layout [128, TILES*D] f16
    table0 = np.zeros((TAB, D), f32)
    table0[tpos] = feature * dis[:, None]
    x0h = (table0.reshape(NC, TILES, 128, D).transpose(0, 2, 1, 3)
           .reshape(NC, 128, TILES * D).astype(np.float16))

    doff_all = doff_all.transpose(0, 2, 1)              # [NC, 128, TOTC]

    # single packed upload per core (one array -> one tunnel transfer)
    in_maps = []
    for c in range(NC):
        blob = np.frombuffer(
            x0h[c].tobytes() + idxw[c].tobytes() + doff_all[c].tobytes() +
            dinv_cols[c].tobytes() + disk_cols[c].tobytes(), np.uint8)
        in_maps.append({"blob": blob.copy()})
    return in_maps, tpos, L_C, H_C, blk_cols


def _build(L_C, H_C, blk_cols):
    from concourse import bass, bacc, tile, mybir
    f32 = mybir.dt.float32
    f16 = mybir.dt.float16
    fp8 = mybir.dt.float8e4
    T = L_C + H_C
    BC = BT * T
    TOTC = TILES * T
    TOT = TOTC * 128

    nc = bacc.Bacc("TRN2", target_bir_lowering=False, debug=False, num_devices=NC)
    # packed input blob: x0h f16 | idx16 i16 | doff8 u8 | dinv f32 | disk f32
    o_x0, n_x0 = 0, 128 * TILES * D * 2
    o_idx, n_idx = o_x0 + n_x0, TOT * 2
    o_doff, n_doff = o_idx + n_idx, 128 * TOTC
    o_dinv, n_dinv = o_doff + n_doff, 128 * TILES * 4
    o_disk, n_disk = o_dinv + n_dinv, 128 * K * TILES * 4
    NBYTES = o_disk + n_disk
    blob_d = nc.dram_tensor("blob", [NBYTES], mybir.dt.uint8,
                            kind="ExternalInput").ap()
    x0h_d = blob_d[o_x0:o_x0 + n_x0].bitcast(f16).rearrange("(p f) -> p f", p=128)
    idx16_d = blob_d[o_idx:o_idx + n_idx].bitcast(mybir.dt.int16) \
                    .rearrange("(p f) -> p f", p=16)
    doff8_d = blob_d[o_doff:o_doff + n_doff].rearrange("(p f) -> p f", p=128)
    dinv_d = blob_d[o_dinv:o_dinv + n_dinv].bitcast(f32) \
                   .rearrange("(p f) -> p f", p=128)
    disk_d = blob_d[o_disk:o_disk + n_disk].bitcast(f32) \
                   .rearrange("(p f) -> p f", p=128)
    y_d = nc.dram_tensor("y", [K * ROWS, D], fp8, kind="ExternalOutput").ap()

    with tile.TileContext(nc) as tc:
        with tc.tile_pool(name="stat", bufs=1) as stat, \
             tc.tile_pool(name="g", bufs=2) as gp, \
             tc.tile_pool(name="s", bufs=2) as sp, \
             tc.tile_pool(name="o", bufs=3) as op_, \
             tc.tile_pool(name="ps", bufs=4, space="PSUM") as ps, \
             tc.tile_pool(name="dram", bufs=2, space="DRAM") as dr:
            idx_sb = stat.tile([128, TOT // 16], mybir.dt.int16)
            for g8 in range(8):
                nc.sync.dma_start(idx_sb[g8 * 16:(g8 + 1) * 16, :], idx16_d[:])
            doff8_sb = stat.tile([128, TOTC], mybir.dt.uint8)
            doff_sb = stat.tile([128, TOTC], f32)
            nc.sync.dma_start(doff8_sb[:], doff8_d[:])
            nc.scalar.copy(doff_sb[:], doff8_sb[:])
            dinv_sb = stat.tile([128, TILES], f32)
            disk_sb = stat.tile([128, K * TILES], f32)
            nc.sync.dma_start(dinv_sb[:], dinv_d[:])
            nc.sync.dma_start(disk_sb[:], disk_d[:])
            j_sb = stat.tile([128, 128], f32)
            nc.gpsimd.iota(j_sb[:], [[1, 128]], channel_multiplier=0,
                           allow_small_or_imprecise_dtypes=True)

            # hop-0 table: upload own rows f16, convert, AllGather
            x0h_sb = stat.tile([128, TILES * D], f16)
            x0_sb = stat.tile([128, TILES * D], f32)
            nc.sync.dma_start(x0h_sb[:], x0h_d[:])
            nc.scalar.copy(x0_sb[:], x0h_sb[:])
            ag_in = dr.tile([ROWS, D], f32, tag="agin")
            for t in range(TILES):
                nc.sync.dma_start(ag_in[t * 128:(t + 1) * 128, :],
                                  x0_sb[:, t * D:(t + 1) * D])
            prev = dr.tile([TAB, D], f32, tag="agout", addr_space="Shared")
            nc.gpsimd.collective_compute(
                "AllGather", mybir.AluOpType.bypass,
                replica_groups=[list(range(NC))],
                ins=[ag_in[:]], outs=[prev[:]])

            for k in range(1, K + 1):
                srctab = prev[:]
                lo_ap = srctab[0:LO_ROWS, :]
                hi_ap = srctab[HI_BASE:TAB, :]
                if k < K:
                    ag_in = dr.tile([ROWS, D], f32, tag="agin")
                GCH = int(os.environ.get("LGCN_GCH", "8"))  # cols per gather instr
                for b in range(NB):
                    g = gp.tile([128, BC, D], f32, tag="g")
                    for half in range(2):
                        c0, nn = blk_cols[b * 2 + half]
                        colbase = 0 if half == 0 else BT * L_C
                        ncols = (BT * L_C) if half == 0 else (BT * H_C)
                        for w0 in range(0, ncols, GCH):
                            wc = min(GCH, ncols - w0)
                            ni = wc * 128
                            nc.gpsimd.dma_gather(
                                out_ap=g[:, colbase + w0:colbase + w0 + wc, :],
                                in_ap=lo_ap if half == 0 else hi_ap,
                                idxs_ap=idx_sb[:, c0 + w0 * 8:c0 + w0 * 8 + ni // 16],
                                num_idxs=ni, num_idxs_reg=ni, elem_size=D,
                            )
                    for ti in range(BT):
                        t = b * BT + ti
                        s = sp.tile([128, T, 128], f32, tag="s")
                        dlo = doff_sb[:, b * BC + ti * L_C:][:, :L_C]
                        dhi = doff_sb[:, b * BC + BT * L_C + ti * H_C:][:, :H_C]
                        nc.vector.tensor_tensor(
                            out=s[:, 0:L_C, :],
                            in0=j_sb[:].unsqueeze(1).broadcast_to([128, L_C, 128]),
                            in1=dlo.unsqueeze(2).broadcast_to([128, L_C, 128]),
                            op=mybir.AluOpType.is_equal)
                        nc.vector.tensor_tensor(
                            out=s[:, L_C:T, :],
                            in0=j_sb[:].unsqueeze(1).broadcast_to([128, H_C, 128]),
                            in1=dhi.unsqueeze(2).broadcast_to([128, H_C, 128]),
                            op=mybir.AluOpType.is_equal)
                        acc = ps.tile([128, D], f32, tag="acc")
                        for j in range(T):
                            col = ti * L_C + j if j < L_C else BT * L_C + ti * H_C + (j - L_C)
                            nc.tensor.matmul(acc[:], s[:, j], g[:, col],
                                             start=(j == 0), stop=(j == T - 1))
                        yt = op_.tile([128, D], fp8, tag="yt")
                        nc.any.tensor_scalar_mul(
                            yt[:], acc[:], disk_sb[:, (k - 1) * TILES + t:
                                                   (k - 1) * TILES + t + 1])
                        nc.sync.dma_start(y_d[(k - 1) * ROWS + t * 128:
                                              (k - 1) * ROWS + (t + 1) * 128, :], yt[:])
                        if k < K:
                            xp = op_.tile([128, D], f32, tag="xp")
                            nc.vector.tensor_scalar_mul(xp[:], acc[:], dinv_sb[:, t:t + 1])
                            nc.sync.dma_start(ag_in[t * 128:(t + 1) * 128, :], xp[:])
                if k < K:
                    ag_out = dr.tile([TAB, D], f32, tag="agout", addr_space="Shared")
                    nc.gpsimd.collective_compute(
                        "AllGather", mybir.AluOpType.bypass,
                        replica_groups=[list(range(NC))],
                        ins=[ag_in[:]], outs=[ag_out[:]])
                    prev = ag_out
    nc.compile()
    return nc


def kernel(feature, edge_index):
    feature = np.asarray(feature, np.float32)
    edge_index = np.asarray(edge_index)
    in_maps, tpos, L_C, H_C, blk_cols = _preprocess(feature, edge_index)
    ck = (L_C, H_C)
    if ck not in _cache:
        _cache[ck] = _build(L_C, H_C, blk_cols)
    nc = _cache[ck]
    from concourse import bass_utils
    import time as _time
    _t0 = _time.time()
    res = bass_utils.run_bass_kernel_spmd(nc, in_maps, core_ids=list(range(NC)))
    global LAST_RUN_S
    LAST_RUN_S = _time.time() - _t0
    y = np.stack([np.asarray(res.results[c]["y"]) for c in range(NC)])
    yf = y.astype(np.float32)                            # [NC, K*ROWS, D]
    Z = np.empty((N, (K + 1) * D), np.float32)
    Z[:, :D] = feature
    for k in range(1, K + 1):
        blk = yf[:, (k - 1) * ROWS:k * ROWS, :].reshape(NC * ROWS, D)
        Z[:, k * D:(k + 1) * D] = blk[tpos] * np.float32(1.0 / S8[k - 1])
    return Z


# revision 12
# speedup vs baseline: 1.0945x; 1.0945x over previous
"""LGCN (K-hop symmetric-normalized graph propagation) on 8 Trainium2 cores.

Algorithm: Z = concat([X, A_hat X, ..., A_hat^K X]) with
A_hat = D^-1/2 (A + I) D^-1/2 (existing self-edges dropped, loops added).

Folding: with dis = deg^-1/2, x'_k = dis * y_k obeys
    x'_{k+1} = dinv * segsum_dst(x'_k[src]),   y_k = x'_k / dis
over the unweighted self-loop-augmented edge list. So each hop is a pure
gather + segment-sum + row-scale: no per-edge weights on device.

Device mapping (SPMD, 8 cores, dst-sharded):
  - x' table [50176, 64] f32 lives in DRAM, rebuilt per hop by AllGather.
    Hop 0's table is also built on device: each core uploads only its own
    6272 rows (f16), converts to f32, and AllGathers - so the host never
    ships the full table 8x.
  - dma_gather (SWDGE) pulls per-edge source rows into SBUF, 128 edges per
    "chunk" (one free column).
  - one-hot S matrices (S^T[e, d] = edge e targets tile-row d) are built on
    DVE with a single is_equal over a broadcast iota row; segment-sum is
    S^T.T @ G on the PE accumulating into PSUM per 128-dst tile.
  - PSUM is scaled by dinv (next-hop x', f32) and by dis*s_k into fp8-e4m3
    y output per tile (per-hop power-of-2 scale s_k keeps values in fp8
    range; host divides back). Hop blocks carry only ~6% of ||Z||^2 (the
    raw-feature block dominates), so fp8 rounding lands at ~7e-3 rel err
    against the 2e-2 gate.
Edges are padded per (tile, src-half) to uniform chunk counts so the
program is identical on all cores (int16 gather indices need a lo/hi
table split at row 25088 / 17408 base).

Host<->device traffic per call (the wall-clock bottleneck over the axon
tunnel): one packed 1.4MB/core input blob + 3.2MB/core donated output
zeros up, 3.2MB/core fp8 y down (~63MB total vs ~330MB for the all-f32
variant). A persistent JAX compilation cache removes the ~1s per-call
XLA rebuild. Device exec is ~6ms for all 8 hops (measured K=1 vs K=8);
the remaining wall is tunnel transfer plus fixed RPC/trace overhead.
"""
import sys
sys.path.insert(0, "/opt/trn_rl_repo")
import math
import os
import numpy as np

os.environ.setdefault("JAX_COMPILATION_CACHE_DIR", "/tmp/jax_comp_cache")
import jax
try:
    jax.config.update("jax_compilation_cache_dir",
                      os.environ["JAX_COMPILATION_CACHE_DIR"])
    jax.config.update("jax_persistent_cache_min_entry_size_bytes", 0)
    jax.config.update("jax_persistent_cache_min_compile_time_secs", 0.0)
except Exception:
    pass

N = 50000
D = 64
K = int(os.environ.get("LGCN_K", "8"))
NC = 8
NSH = N // NC            # 6250 nodes per core
TILES = 49               # 128-dst tiles per core
ROWS = TILES * 128       # 6272 padded rows per core
TAB = NC * ROWS          # 50176 table rows
THRESH = 25088           # src rows below -> lo gather
HI_BASE = 17408          # hi gather table base (TAB - HI_BASE = 32768 rows)
LO_ROWS = 32768
BT = 7                   # tiles per gather batch
NB = TILES // BT         # 7 batches
# per-hop power-of-2 fp8 output scales (y_k*s_k ~ N(0, 4..10), absmax <~110
# vs e4m3 max 240). Hop stds decay 0.25 -> 0.005 then plateau.
S8 = [32.0, 64.0, 256.0, 512.0, 1024.0, 2048.0, 2048.0, 2048.0][:K] + \
     [2048.0] * max(0, K - 8)

_cache = {}
LAST_RUN_S = None


def _preprocess(feature, edge_index):
    f32 = np.float32
    src = edge_index[0].astype(np.int64)
    dst = edge_index[1].astype(np.int64)
    keep = src != dst
    ks, kd = src[keep], dst[keep]
    deg = (np.bincount(ks, minlength=N) + 1).astype(f32)
    dis = (1.0 / np.sqrt(deg)).astype(f32)
    dinv = (dis * dis).astype(f32)

    # balanced node -> (tile, row) assignment per core, by in-degree
    indeg = np.bincount(kd, minlength=N) + 1
    tile_of = np.empty(N, np.int32)
    row_of = np.empty(N, np.int32)
    for c in range(NC):
        nodes = np.arange(c * NSH, (c + 1) * NSH)
        order = nodes[np.argsort(-indeg[nodes], kind="stable")]
        loads = np.zeros(TILES, np.int64)
        counts = np.zeros(TILES, np.int64)
        for n in order:
            cand = np.where(counts < 128, loads, 1 << 60)
            t = int(np.argmin(cand))
            tile_of[n] = t
            row_of[n] = counts[t]
            counts[t] += 1
            loads[t] += indeg[n]
    core_of = (np.arange(N) // NSH).astype(np.int32)
    tpos = core_of * ROWS + tile_of * 128 + row_of      # table position per node

    # augmented edge list (kept edges + self loops), dst-sharded
    es = np.concatenate([ks, np.arange(N)])
    ed = np.concatenate([kd, np.arange(N)])
    srcr = tpos[es].astype(np.int64)
    ecore = core_of[ed]
    etile = tile_of[ed].astype(np.int64)
    erow = row_of[ed].astype(np.int64)
    lo = srcr < THRESH

    # group edges by (core, tile, half); rank within group
    key = (ecore * TILES + etile) * 2 + (~lo)
    order = np.argsort(key, kind="stable")
    skey = key[order]
    counts = np.bincount(skey, minlength=NC * TILES * 2)
    starts = np.concatenate([[0], np.cumsum(counts)[:-1]])
    rank = np.arange(len(order)) - starts[skey]

    L_C = int(math.ceil(counts[0::2].max() / 128))
    H_C = int(math.ceil(counts[1::2].max() / 128))
    T = L_C + H_C
    BC = BT * T                                         # G cols per batch
    TOTC = TILES * T
    TOT = TOTC * 128

    # slot number for each edge (per core)
    sk = skey
    score = sk // (TILES * 2)
    st = (sk // 2) % TILES
    shalf = sk % 2
    b = st // BT
    ti = st % BT
    chunk = rank // 128
    pos = rank % 128
    col_in_batch = np.where(shalf == 0, ti * L_C + chunk,
                            BT * L_C + ti * H_C + chunk)
    col = b * BC + col_in_batch
    slot = col * 128 + pos

    sidx = np.where(shalf == 0, srcr[order], srcr[order] - HI_BASE).astype(np.int16)
    sdoff = erow[order].astype(np.uint8)

    idx_all = np.zeros((NC, TOT), np.int16)
    doff_all = np.full((NC, TOTC, 128), 255, np.uint8)
    idx_all[score, slot] = sidx
    doff_all[score, col, pos] = sdoff

    # wrap idx per gather block (block = batch x half, contiguous slots);
    # single 16-row copy - the kernel replicates it to 128 partitions.
    lo_n = BT * L_C * 128
    hi_n = BT * H_C * 128
    idxw = np.empty((NC, 16, TOT // 16), np.int16)
    blk_cols = []
    off = 0
    for bb in range(NB):
        for half, nn in ((0, lo_n), (1, hi_n)):
            blk = idx_all[:, off:off + nn]              # [NC, nn]
            w = blk.reshape(NC, nn // 16, 16).transpose(0, 2, 1)  # [NC,16,nn/16]
            c0 = off // 16
            idxw[:, :, c0:c0 + nn // 16] = w
            blk_cols.append((c0, nn))
            off += nn

    # per-tile scale columns: dinv [128, TILES]; dis scaled per hop
    # [128, K*TILES] (hop k uses cols [(k-1)*TILES, k*TILES)); f16 on the
    # wire, converted to f32 on device (scale rounding ~2e-4, far below the
    # fp8 output noise)
    dinv_cols = np.zeros((NC, 128, TILES), f32)
    dis_cols = np.zeros((NC, 128, TILES), f32)
    dinv_cols[core_of, row_of, tile_of] = dinv
    dis_cols[core_of, row_of, tile_of] = dis
    disk_cols = np.concatenate([dis_cols * s for s in S8], axis=2)
    dinv_cols = dinv_cols.astype(np.float16)
    disk_cols = disk_cols.astype(np.float16)

    # per-core x'_0 rows in SBUF layout [128, TILES*D] fp8 (sim-verified:
    # fp8 x0 lifts total rel err 6.7e-3 -> 9.4e-3 vs the 2e-2 gate)
    import ml_dtypes
    table0 = np.zeros((TAB, D), f32)
    table0[tpos] = feature * dis[:, None]
    x0h = (table0.reshape(NC, TILES, 128, D).transpose(0, 2, 1, 3)
           .reshape(NC, 128, TILES * D).astype(ml_dtypes.float8_e4m3))

    doff_all = doff_all.transpose(0, 2, 1)              # [NC, 128, TOTC]

    # single packed upload per core (one array -> one tunnel transfer)
    in_maps = []
    for c in range(NC):
        blob = np.frombuffer(
            x0h[c].tobytes() + idxw[c].tobytes() + doff_all[c].tobytes() +
            dinv_cols[c].tobytes() + disk_cols[c].tobytes(), np.uint8)
        in_maps.append({"blob": blob.copy()})
    return in_maps, tpos, L_C, H_C, blk_cols


def _build(L_C, H_C, blk_cols):
    from concourse import bass, bacc, tile, mybir
    f32 = mybir.dt.float32
    f16 = mybir.dt.float16
    fp8 = mybir.dt.float8e4
    T = L_C + H_C
    BC = BT * T
    TOTC = TILES * T
    TOT = TOTC * 128

    nc = bacc.Bacc("TRN2", target_bir_lowering=False, debug=False, num_devices=NC)
    # packed input blob: x0h fp8 | idx16 i16 | doff8 u8 | dinv f16 | disk f16
    o_x0, n_x0 = 0, 128 * TILES * D
    o_idx, n_idx = o_x0 + n_x0, TOT * 2
    o_doff, n_doff = o_idx + n_idx, 128 * TOTC
    o_dinv, n_dinv = o_doff + n_doff, 128 * TILES * 2
    o_disk, n_disk = o_dinv + n_dinv, 128 * K * TILES * 2
    NBYTES = o_disk + n_disk
    blob_d = nc.dram_tensor("blob", [NBYTES], mybir.dt.uint8,
                            kind="ExternalInput").ap()
    x0h_d = blob_d[o_x0:o_x0 + n_x0].bitcast(fp8).rearrange("(p f) -> p f", p=128)
    idx16_d = blob_d[o_idx:o_idx + n_idx].bitcast(mybir.dt.int16) \
                    .rearrange("(p f) -> p f", p=16)
    doff8_d = blob_d[o_doff:o_doff + n_doff].rearrange("(p f) -> p f", p=128)
    dinv_d = blob_d[o_dinv:o_dinv + n_dinv].bitcast(f16) \
                   .rearrange("(p f) -> p f", p=128)
    disk_d = blob_d[o_disk:o_disk + n_disk].bitcast(f16) \
                   .rearrange("(p f) -> p f", p=128)
    # y declared uint8 (bitcast to fp8 at the DMA): dodges any fp8-specific
    # handling on the host fetch path; bytes are identical
    y_d = nc.dram_tensor("y", [K * ROWS, D], mybir.dt.uint8,
                         kind="ExternalOutput").ap()

    with tile.TileContext(nc) as tc:
        with tc.tile_pool(name="stat", bufs=1) as stat, \
             tc.tile_pool(name="g", bufs=2) as gp, \
             tc.tile_pool(name="s", bufs=2) as sp, \
             tc.tile_pool(name="o", bufs=3) as op_, \
             tc.tile_pool(name="ps", bufs=4, space="PSUM") as ps, \
             tc.tile_pool(name="dram", bufs=2, space="DRAM") as dr:
            idx_sb = stat.tile([128, TOT // 16], mybir.dt.int16)
            for g8 in range(8):
                nc.sync.dma_start(idx_sb[g8 * 16:(g8 + 1) * 16, :], idx16_d[:])
            doff8_sb = stat.tile([128, TOTC], mybir.dt.uint8)
            doff_sb = stat.tile([128, TOTC], f32)
            nc.sync.dma_start(doff8_sb[:], doff8_d[:])
            nc.scalar.copy(doff_sb[:], doff8_sb[:])
            dinv16_sb = stat.tile([128, TILES], f16)
            disk16_sb = stat.tile([128, K * TILES], f16)
            nc.sync.dma_start(dinv16_sb[:], dinv_d[:])
            nc.sync.dma_start(disk16_sb[:], disk_d[:])
            dinv_sb = stat.tile([128, TILES], f32)
            disk_sb = stat.tile([128, K * TILES], f32)
            nc.scalar.copy(dinv_sb[:], dinv16_sb[:])
            nc.scalar.copy(disk_sb[:], disk16_sb[:])
            j_sb = stat.tile([128, 128], f32)
            nc.gpsimd.iota(j_sb[:], [[1, 128]], channel_multiplier=0,
                           allow_small_or_imprecise_dtypes=True)

            # hop-0 table: upload own rows fp8, convert, AllGather
            x0h_sb = stat.tile([128, TILES * D], fp8)
            x0_sb = stat.tile([128, TILES * D], f32)
            nc.sync.dma_start(x0h_sb[:], x0h_d[:])
            nc.scalar.copy(x0_sb[:], x0h_sb[:])
            ag_in = dr.tile([ROWS, D], f32, tag="agin")
            for t in range(TILES):
                nc.sync.dma_start(ag_in[t * 128:(t + 1) * 128, :],
                                  x0_sb[:, t * D:(t + 1) * D])
            prev = dr.tile([TAB, D], f32, tag="agout", addr_space="Shared")
            nc.gpsimd.collective_compute(
                "AllGather", mybir.AluOpType.bypass,
                replica_groups=[list(range(NC))],
                ins=[ag_in[:]], outs=[prev[:]])

            for k in range(1, K + 1):
                srctab = prev[:]
                lo_ap = srctab[0:LO_ROWS, :]
                hi_ap = srctab[HI_BASE:TAB, :]
                if k < K:
                    ag_in = dr.tile([ROWS, D], f32, tag="agin")
                GCH = int(os.environ.get("LGCN_GCH", "8"))  # cols per gather instr
                for b in range(NB):
                    g = gp.tile([128, BC, D], f32, tag="g")
                    for half in range(2):
                        c0, nn = blk_cols[b * 2 + half]
                        colbase = 0 if half == 0 else BT * L_C
                        ncols = (BT * L_C) if half == 0 else (BT * H_C)
                        for w0 in range(0, ncols, GCH):
                            wc = min(GCH, ncols - w0)
                            ni = wc * 128
                            nc.gpsimd.dma_gather(
                                out_ap=g[:, colbase + w0:colbase + w0 + wc, :],
                                in_ap=lo_ap if half == 0 else hi_ap,
                                idxs_ap=idx_sb[:, c0 + w0 * 8:c0 + w0 * 8 + ni // 16],
                                num_idxs=ni, num_idxs_reg=ni, elem_size=D,
                            )
                    for ti in range(BT):
                        t = b * BT + ti
                        s = sp.tile([128, T, 128], f32, tag="s")
                        dlo = doff_sb[:, b * BC + ti * L_C:][:, :L_C]
                        dhi = doff_sb[:, b * BC + BT * L_C + ti * H_C:][:, :H_C]
                        nc.vector.tensor_tensor(
                            out=s[:, 0:L_C, :],
                            in0=j_sb[:].unsqueeze(1).broadcast_to([128, L_C, 128]),
                            in1=dlo.unsqueeze(2).broadcast_to([128, L_C, 128]),
                            op=mybir.AluOpType.is_equal)
                        nc.vector.tensor_tensor(
                            out=s[:, L_C:T, :],
                            in0=j_sb[:].unsqueeze(1).broadcast_to([128, H_C, 128]),
                            in1=dhi.unsqueeze(2).broadcast_to([128, H_C, 128]),
                            op=mybir.AluOpType.is_equal)
                        acc = ps.tile([128, D], f32, tag="acc")
                        for j in range(T):
                            col = ti * L_C + j if j < L_C else BT * L_C + ti * H_C + (j - L_C)
                            nc.tensor.matmul(acc[:], s[:, j], g[:, col],
                                             start=(j == 0), stop=(j == T - 1))
                        yt = op_.tile([128, D], fp8, tag="yt")
                        nc.any.tensor_scalar_mul(
                            yt[:], acc[:], disk_sb[:, (k - 1) * TILES + t:
                                                   (k - 1) * TILES + t + 1])
                        nc.sync.dma_start(
                            y_d[(k - 1) * ROWS + t * 128:
                                (k - 1) * ROWS + (t + 1) * 128, :].bitcast(fp8),
                            yt[:])
                        if k < K:
                            xp = op_.tile([128, D], f32, tag="xp")
                            nc.vector.tensor_scalar_mul(xp[:], acc[:], dinv_sb[:, t:t + 1])
                            nc.sync.dma_start(ag_in[t * 128:(t + 1) * 128, :], xp[:])
                if k < K:
                    ag_out = dr.tile([TAB, D], f32, tag="agout", addr_space="Shared")
                    nc.gpsimd.collective_compute(
                        "AllGather", mybir.AluOpType.bypass,
                        replica_groups=[list(range(NC))],
                        ins=[ag_in[:]], outs=[ag_out[:]])
                    prev = ag_out
    nc.compile()
    return nc


def kernel(feature, edge_index):
    feature = np.asarray(feature, np.float32)
    edge_index = np.asarray(edge_index)
    in_maps, tpos, L_C, H_C, blk_cols = _preprocess(feature, edge_index)
    ck = (L_C, H_C)
    if ck not in _cache:
        _cache[ck] = _build(L_C, H_C, blk_cols)
    nc = _cache[ck]
    from concourse import bass_utils
    import time as _time
    _t0 = _time.time()
    res = bass_utils.run_bass_kernel_spmd(nc, in_maps, core_ids=list(range(NC)))
    global LAST_RUN_S
    LAST_RUN_S = _time.time() - _t0
    import ml_dtypes
    y = np.stack([np.asarray(res.results[c]["y"]) for c in range(NC)])
    yf = y.view(ml_dtypes.float8_e4m3).astype(np.float32)  # [NC, K*ROWS, D]
    Z = np.empty((N, (K + 1) * D), np.float32)
    Z[:, :D] = feature
    for k in range(1, K + 1):
        blk = yf[:, (k - 1) * ROWS:k * ROWS, :].reshape(NC * ROWS, D)
        Z[:, k * D:(k + 1) * D] = blk[tpos] * np.float32(1.0 / S8[k - 1])
    return Z


# revision 13
# speedup vs baseline: 1.2207x; 1.1153x over previous
"""LGCN (K-hop symmetric-normalized graph propagation) on 8 Trainium2 cores.

Algorithm: Z = concat([X, A_hat X, ..., A_hat^K X]) with
A_hat = D^-1/2 (A + I) D^-1/2 (existing self-edges dropped, loops added).

Folding: with dis = deg^-1/2, x'_k = dis * y_k obeys
    x'_{k+1} = dinv * segsum_dst(x'_k[src]),   y_k = x'_k / dis
over the unweighted self-loop-augmented edge list. So each hop is a pure
gather + segment-sum + row-scale: no per-edge weights on device.

Device mapping (SPMD, 8 cores, dst-sharded):
  - x' table [50176, 64] f32 lives in DRAM, rebuilt per hop by AllGather.
    Hop 0's table is also built on device: each core uploads only its own
    6272 rows (f16), converts to f32, and AllGathers - so the host never
    ships the full table 8x.
  - dma_gather (SWDGE) pulls per-edge source rows into SBUF, 128 edges per
    "chunk" (one free column).
  - one-hot S matrices (S^T[e, d] = edge e targets tile-row d) are built on
    DVE with a single is_equal over a broadcast iota row; segment-sum is
    S^T.T @ G on the PE accumulating into PSUM per 128-dst tile.
  - PSUM is scaled by dinv (next-hop x', f32) and by dis*s_k into fp8-e4m3
    y output per tile (per-hop power-of-2 scale s_k keeps values in fp8
    range; host divides back). Hop blocks carry only ~6% of ||Z||^2 (the
    raw-feature block dominates), so fp8 y + fp8 x0 upload land at 9.4e-3
    rel err (sim-verified, matches hardware bit-for-bit) vs the 2e-2 gate.
Edges are padded per (tile, src-half) to uniform chunk counts so the
program is identical on all cores (int16 gather indices need a lo/hi
table split at row 25088 / 17408 base).

Host<->device traffic per call (the wall-clock bottleneck over the axon
tunnel, ~55-110MB/s): one packed 0.89MB/core input blob (fp8 x0, i16 idx,
u8 dst-rows, f16 scales) + 3.2MB/core donated output zeros up, 3.2MB/core
y down (~58MB total vs ~330MB for the all-f32 variant). y is declared
uint8 and bitcast to fp8 at the DMA so the host fetch stays on the plain
byte path. A persistent JAX compilation cache removes the ~1s per-call
XLA rebuild. Device exec is ~6ms for all 8 hops (measured K=1 vs K=8);
the remaining wall is tunnel transfer plus fixed RPC/trace overhead.
"""
import sys
sys.path.insert(0, "/opt/trn_rl_repo")
import math
import os
import numpy as np

os.environ.setdefault("JAX_COMPILATION_CACHE_DIR", "/tmp/jax_comp_cache")
import jax
try:
    jax.config.update("jax_compilation_cache_dir",
                      os.environ["JAX_COMPILATION_CACHE_DIR"])
    jax.config.update("jax_persistent_cache_min_entry_size_bytes", 0)
    jax.config.update("jax_persistent_cache_min_compile_time_secs", 0.0)
except Exception:
    pass

N = 50000
D = 64
K = int(os.environ.get("LGCN_K", "8"))
NC = 8
NSH = N // NC            # 6250 nodes per core
TILES = 49               # 128-dst tiles per core
ROWS = TILES * 128       # 6272 padded rows per core
TAB = NC * ROWS          # 50176 table rows
THRESH = 25088           # src rows below -> lo gather
HI_BASE = 17408          # hi gather table base (TAB - HI_BASE = 32768 rows)
LO_ROWS = 32768
BT = 7                   # tiles per gather batch
NB = TILES // BT         # 7 batches
# per-hop power-of-2 fp8 output scales (y_k*s_k ~ N(0, 4..10), absmax <~110
# vs e4m3 max 240). Hop stds decay 0.25 -> 0.005 then plateau.
S8 = [32.0, 64.0, 256.0, 512.0, 1024.0, 2048.0, 2048.0, 2048.0][:K] + \
     [2048.0] * max(0, K - 8)

_cache = {}
LAST_RUN_S = None


def _preprocess(feature, edge_index):
    f32 = np.float32
    src = edge_index[0].astype(np.int64)
    dst = edge_index[1].astype(np.int64)
    keep = src != dst
    ks, kd = src[keep], dst[keep]
    deg = (np.bincount(ks, minlength=N) + 1).astype(f32)
    dis = (1.0 / np.sqrt(deg)).astype(f32)
    dinv = (dis * dis).astype(f32)

    # balanced node -> (tile, row) assignment per core, by in-degree
    indeg = np.bincount(kd, minlength=N) + 1
    tile_of = np.empty(N, np.int32)
    row_of = np.empty(N, np.int32)
    for c in range(NC):
        nodes = np.arange(c * NSH, (c + 1) * NSH)
        order = nodes[np.argsort(-indeg[nodes], kind="stable")]
        loads = np.zeros(TILES, np.int64)
        counts = np.zeros(TILES, np.int64)
        for n in order:
            cand = np.where(counts < 128, loads, 1 << 60)
            t = int(np.argmin(cand))
            tile_of[n] = t
            row_of[n] = counts[t]
            counts[t] += 1
            loads[t] += indeg[n]
    core_of = (np.arange(N) // NSH).astype(np.int32)
    tpos = core_of * ROWS + tile_of * 128 + row_of      # table position per node

    # augmented edge list (kept edges + self loops), dst-sharded
    es = np.concatenate([ks, np.arange(N)])
    ed = np.concatenate([kd, np.arange(N)])
    srcr = tpos[es].astype(np.int64)
    ecore = core_of[ed]
    etile = tile_of[ed].astype(np.int64)
    erow = row_of[ed].astype(np.int64)
    lo = srcr < THRESH

    # group edges by (core, tile, half); rank within group
    key = (ecore * TILES + etile) * 2 + (~lo)
    order = np.argsort(key, kind="stable")
    skey = key[order]
    counts = np.bincount(skey, minlength=NC * TILES * 2)
    starts = np.concatenate([[0], np.cumsum(counts)[:-1]])
    rank = np.arange(len(order)) - starts[skey]

    L_C = int(math.ceil(counts[0::2].max() / 128))
    H_C = int(math.ceil(counts[1::2].max() / 128))
    T = L_C + H_C
    BC = BT * T                                         # G cols per batch
    TOTC = TILES * T
    TOT = TOTC * 128

    # slot number for each edge (per core)
    sk = skey
    score = sk // (TILES * 2)
    st = (sk // 2) % TILES
    shalf = sk % 2
    b = st // BT
    ti = st % BT
    chunk = rank // 128
    pos = rank % 128
    col_in_batch = np.where(shalf == 0, ti * L_C + chunk,
                            BT * L_C + ti * H_C + chunk)
    col = b * BC + col_in_batch
    slot = col * 128 + pos

    sidx = np.where(shalf == 0, srcr[order], srcr[order] - HI_BASE).astype(np.int16)
    sdoff = erow[order].astype(np.uint8)

    idx_all = np.zeros((NC, TOT), np.int16)
    doff_all = np.full((NC, TOTC, 128), 255, np.uint8)
    idx_all[score, slot] = sidx
    doff_all[score, col, pos] = sdoff

    # wrap idx per gather block (block = batch x half, contiguous slots);
    # single 16-row copy - the kernel replicates it to 128 partitions.
    lo_n = BT * L_C * 128
    hi_n = BT * H_C * 128
    idxw = np.empty((NC, 16, TOT // 16), np.int16)
    blk_cols = []
    off = 0
    for bb in range(NB):
        for half, nn in ((0, lo_n), (1, hi_n)):
            blk = idx_all[:, off:off + nn]              # [NC, nn]
            w = blk.reshape(NC, nn // 16, 16).transpose(0, 2, 1)  # [NC,16,nn/16]
            c0 = off // 16
            idxw[:, :, c0:c0 + nn // 16] = w
            blk_cols.append((c0, nn))
            off += nn

    # per-tile scale columns: dinv [128, TILES]; dis scaled per hop
    # [128, K*TILES] (hop k uses cols [(k-1)*TILES, k*TILES)); f16 on the
    # wire, converted to f32 on device (scale rounding ~2e-4, far below the
    # fp8 output noise)
    dinv_cols = np.zeros((NC, 128, TILES), f32)
    dis_cols = np.zeros((NC, 128, TILES), f32)
    dinv_cols[core_of, row_of, tile_of] = dinv
    dis_cols[core_of, row_of, tile_of] = dis
    disk_cols = np.concatenate([dis_cols * s for s in S8], axis=2)
    dinv_cols = dinv_cols.astype(np.float16)
    disk_cols = disk_cols.astype(np.float16)

    # per-core x'_0 rows in SBUF layout [128, TILES*D] fp8 (sim-verified:
    # fp8 x0 lifts total rel err 6.7e-3 -> 9.4e-3 vs the 2e-2 gate)
    import ml_dtypes
    table0 = np.zeros((TAB, D), f32)
    table0[tpos] = feature * dis[:, None]
    x0h = (table0.reshape(NC, TILES, 128, D).transpose(0, 2, 1, 3)
           .reshape(NC, 128, TILES * D).astype(ml_dtypes.float8_e4m3))

    doff_all = doff_all.transpose(0, 2, 1)              # [NC, 128, TOTC]

    # single packed upload per core (one array -> one tunnel transfer)
    in_maps = []
    for c in range(NC):
        blob = np.frombuffer(
            x0h[c].tobytes() + idxw[c].tobytes() + doff_all[c].tobytes() +
            dinv_cols[c].tobytes() + disk_cols[c].tobytes(), np.uint8)
        in_maps.append({"blob": blob.copy()})
    return in_maps, tpos, L_C, H_C, blk_cols


def _build(L_C, H_C, blk_cols):
    from concourse import bass, bacc, tile, mybir
    f32 = mybir.dt.float32
    f16 = mybir.dt.float16
    fp8 = mybir.dt.float8e4
    T = L_C + H_C
    BC = BT * T
    TOTC = TILES * T
    TOT = TOTC * 128

    nc = bacc.Bacc("TRN2", target_bir_lowering=False, debug=False, num_devices=NC)
    # packed input blob: x0h fp8 | idx16 i16 | doff8 u8 | dinv f16 | disk f16
    o_x0, n_x0 = 0, 128 * TILES * D
    o_idx, n_idx = o_x0 + n_x0, TOT * 2
    o_doff, n_doff = o_idx + n_idx, 128 * TOTC
    o_dinv, n_dinv = o_doff + n_doff, 128 * TILES * 2
    o_disk, n_disk = o_dinv + n_dinv, 128 * K * TILES * 2
    NBYTES = o_disk + n_disk
    blob_d = nc.dram_tensor("blob", [NBYTES], mybir.dt.uint8,
                            kind="ExternalInput").ap()
    x0h_d = blob_d[o_x0:o_x0 + n_x0].bitcast(fp8).rearrange("(p f) -> p f", p=128)
    idx16_d = blob_d[o_idx:o_idx + n_idx].bitcast(mybir.dt.int16) \
                    .rearrange("(p f) -> p f", p=16)
    doff8_d = blob_d[o_doff:o_doff + n_doff].rearrange("(p f) -> p f", p=128)
    dinv_d = blob_d[o_dinv:o_dinv + n_dinv].bitcast(f16) \
                   .rearrange("(p f) -> p f", p=128)
    disk_d = blob_d[o_disk:o_disk + n_disk].bitcast(f16) \
                   .rearrange("(p f) -> p f", p=128)
    # y declared uint8 (bitcast to fp8 at the DMA): dodges any fp8-specific
    # handling on the host fetch path; bytes are identical
    y_d = nc.dram_tensor("y", [K * ROWS, D], mybir.dt.uint8,
                         kind="ExternalOutput").ap()

    with tile.TileContext(nc) as tc:
        with tc.tile_pool(name="stat", bufs=1) as stat, \
             tc.tile_pool(name="g", bufs=2) as gp, \
             tc.tile_pool(name="s", bufs=2) as sp, \
             tc.tile_pool(name="o", bufs=3) as op_, \
             tc.tile_pool(name="ps", bufs=4, space="PSUM") as ps, \
             tc.tile_pool(name="dram", bufs=2, space="DRAM") as dr:
            idx_sb = stat.tile([128, TOT // 16], mybir.dt.int16)
            for g8 in range(8):
                nc.sync.dma_start(idx_sb[g8 * 16:(g8 + 1) * 16, :], idx16_d[:])
            doff8_sb = stat.tile([128, TOTC], mybir.dt.uint8)
            doff_sb = stat.tile([128, TOTC], f32)
            nc.sync.dma_start(doff8_sb[:], doff8_d[:])
            nc.scalar.copy(doff_sb[:], doff8_sb[:])
            dinv16_sb = stat.tile([128, TILES], f16)
            disk16_sb = stat.tile([128, K * TILES], f16)
            nc.sync.dma_start(dinv16_sb[:], dinv_d[:])
            nc.sync.dma_start(disk16_sb[:], disk_d[:])
            dinv_sb = stat.tile([128, TILES], f32)
            disk_sb = stat.tile([128, K * TILES], f32)
            nc.scalar.copy(dinv_sb[:], dinv16_sb[:])
            nc.scalar.copy(disk_sb[:], disk16_sb[:])
            j_sb = stat.tile([128, 128], f32)
            nc.gpsimd.iota(j_sb[:], [[1, 128]], channel_multiplier=0,
                           allow_small_or_imprecise_dtypes=True)

            # hop-0 table: upload own rows fp8, convert, AllGather
            x0h_sb = stat.tile([128, TILES * D], fp8)
            x0_sb = stat.tile([128, TILES * D], f32)
            nc.sync.dma_start(x0h_sb[:], x0h_d[:])
            nc.scalar.copy(x0_sb[:], x0h_sb[:])
            ag_in = dr.tile([ROWS, D], f32, tag="agin")
            for t in range(TILES):
                nc.sync.dma_start(ag_in[t * 128:(t + 1) * 128, :],
                                  x0_sb[:, t * D:(t + 1) * D])
            prev = dr.tile([TAB, D], f32, tag="agout", addr_space="Shared")
            nc.gpsimd.collective_compute(
                "AllGather", mybir.AluOpType.bypass,
                replica_groups=[list(range(NC))],
                ins=[ag_in[:]], outs=[prev[:]])

            for k in range(1, K + 1):
                srctab = prev[:]
                lo_ap = srctab[0:LO_ROWS, :]
                hi_ap = srctab[HI_BASE:TAB, :]
                if k < K:
                    ag_in = dr.tile([ROWS, D], f32, tag="agin")
                GCH = int(os.environ.get("LGCN_GCH", "8"))  # cols per gather instr
                for b in range(NB):
                    g = gp.tile([128, BC, D], f32, tag="g")
                    for half in range(2):
                        c0, nn = blk_cols[b * 2 + half]
                        colbase = 0 if half == 0 else BT * L_C
                        ncols = (BT * L_C) if half == 0 else (BT * H_C)
                        for w0 in range(0, ncols, GCH):
                            wc = min(GCH, ncols - w0)
                            ni = wc * 128
                            nc.gpsimd.dma_gather(
                                out_ap=g[:, colbase + w0:colbase + w0 + wc, :],
                                in_ap=lo_ap if half == 0 else hi_ap,
                                idxs_ap=idx_sb[:, c0 + w0 * 8:c0 + w0 * 8 + ni // 16],
                                num_idxs=ni, num_idxs_reg=ni, elem_size=D,
                            )
                    for ti in range(BT):
                        t = b * BT + ti
                        s = sp.tile([128, T, 128], f32, tag="s")
                        dlo = doff_sb[:, b * BC + ti * L_C:][:, :L_C]
                        dhi = doff_sb[:, b * BC + BT * L_C + ti * H_C:][:, :H_C]
                        nc.vector.tensor_tensor(
                            out=s[:, 0:L_C, :],
                            in0=j_sb[:].unsqueeze(1).broadcast_to([128, L_C, 128]),
                            in1=dlo.unsqueeze(2).broadcast_to([128, L_C, 128]),
                            op=mybir.AluOpType.is_equal)
                        nc.vector.tensor_tensor(
                            out=s[:, L_C:T, :],
                            in0=j_sb[:].unsqueeze(1).broadcast_to([128, H_C, 128]),
                            in1=dhi.unsqueeze(2).broadcast_to([128, H_C, 128]),
                            op=mybir.AluOpType.is_equal)
                        acc = ps.tile([128, D], f32, tag="acc")
                        for j in range(T):
                            col = ti * L_C + j if j < L_C else BT * L_C + ti * H_C + (j - L_C)
                            nc.tensor.matmul(acc[:], s[:, j], g[:, col],
                                             start=(j == 0), stop=(j == T - 1))
                        yt = op_.tile([128, D], fp8, tag="yt")
                        nc.any.tensor_scalar_mul(
                            yt[:], acc[:], disk_sb[:, (k - 1) * TILES + t:
                                                   (k - 1) * TILES + t + 1])
                        nc.sync.dma_start(
                            y_d[(k - 1) * ROWS + t * 128:
                                (k - 1) * ROWS + (t + 1) * 128, :].bitcast(fp8),
                            yt[:])
                        if k < K:
                            xp = op_.tile([128, D], f32, tag="xp")
                            nc.vector.tensor_scalar_mul(xp[:], acc[:], dinv_sb[:, t:t + 1])
                            nc.sync.dma_start(ag_in[t * 128:(t + 1) * 128, :], xp[:])
                if k < K:
                    ag_out = dr.tile([TAB, D], f32, tag="agout", addr_space="Shared")
                    nc.gpsimd.collective_compute(
                        "AllGather", mybir.AluOpType.bypass,
                        replica_groups=[list(range(NC))],
                        ins=[ag_in[:]], outs=[ag_out[:]])
                    prev = ag_out
    nc.compile()
    return nc


def kernel(feature, edge_index):
    feature = np.asarray(feature, np.float32)
    edge_index = np.asarray(edge_index)
    in_maps, tpos, L_C, H_C, blk_cols = _preprocess(feature, edge_index)
    ck = (L_C, H_C)
    if ck not in _cache:
        _cache[ck] = _build(L_C, H_C, blk_cols)
    nc = _cache[ck]
    from concourse import bass_utils
    import time as _time
    _t0 = _time.time()
    res = bass_utils.run_bass_kernel_spmd(nc, in_maps, core_ids=list(range(NC)))
    global LAST_RUN_S
    LAST_RUN_S = _time.time() - _t0
    import ml_dtypes
    y = np.stack([np.asarray(res.results[c]["y"]) for c in range(NC)])
    yf = y.view(ml_dtypes.float8_e4m3).astype(np.float32)  # [NC, K*ROWS, D]
    Z = np.empty((N, (K + 1) * D), np.float32)
    Z[:, :D] = feature
    for k in range(1, K + 1):
        blk = yf[:, (k - 1) * ROWS:k * ROWS, :].reshape(NC * ROWS, D)
        Z[:, k * D:(k + 1) * D] = blk[tpos] * np.float32(1.0 / S8[k - 1])
    return Z
